# revision 1
# baseline (speedup 1.0000x reference)
"""Trainium2 Bass kernel for nn_Attention, v2.

Sharding: 4 head-groups (3 heads) x 2 query halves (2048 q) = 8 cores.
Each core: K/V for its heads over all 4096 tokens, attention for its
2048 queries, partial output projection; host sums partials + bias.

vs v1 baseline:
- every 64-wide matmul is emitted as an adjacent pair on disjoint PE
  row/col groups (runs concurrently in the 128x128 array): scores use
  kTab chunk-pairs stacked on partitions with qTab duplicated on both
  halves; K/Q projections col-tile into psum halves.
- softmax exp is split between ScalarE (exact table exp) and VectorE
  (Schraudolph int16 fast-exp: round(x*a+b) bitcast as bf16), keeping
  both engines under the PE's busy time.
- PSUM: 5-bank score ring (3+2 chunk sweeps), 2 pv banks, 1 background
  bank for K/Q/V/proj partials.
"""

import os
from collections import deque

import numpy as np
import ml_dtypes

import concourse.bass as bass
import concourse.tile as tile
from concourse import bacc, mybir
from concourse import bass_utils

BF16 = mybir.dt.bfloat16
F32 = mybir.dt.float32
I16 = mybir.dt.int16
NPBF16 = ml_dtypes.bfloat16

N_TOK = 4096
DIM = 768
H = 12
HD = 64
N_CORES = 8
HPC = 3
NQ = N_TOK // 2
CCH = DIM // 128
KCH = N_TOK // 128
NPAIR = KCH // 2
QB = NQ // 512
SCALE = HD ** -0.5

# Schraudolph fast-exp: bf16(int16(round(x * A + B))) ~= exp(x)
SCH_A = 184.6649652337873   # 2**7 / ln 2
SCH_B = 16250.0             # 127 * 2**7 - 6 (mean-error-centering shift)

# sweep plan: chunk start/size pairs covering 32 chunks, alternating 3/2
_SWEEPS = []
_c = 0
while _c < KCH:
    n = 3 if (len(_SWEEPS) % 2 == 0 and _c + 3 <= KCH) else min(2, KCH - _c)
    _SWEEPS.append((_c, n))
    _c += n

_cache = {}


def _build_program(repeat=1, dve_every=5, reload_inputs=True):
    nc = bacc.Bacc(
        "TRN2",
        target_bir_lowering=False,
        debug=False,
        enable_asserts=False,
        num_devices=N_CORES,
    )

    xw_d = nc.dram_tensor("xw", [128, CCH, N_TOK], BF16, kind="ExternalInput").ap()
    qx_d = nc.dram_tensor("qx", [128, CCH, NQ], BF16, kind="ExternalInput").ap()
    wk_d = nc.dram_tensor("wk", [128, CCH, HPC * HD], BF16, kind="ExternalInput").ap()
    wq_d = nc.dram_tensor("wq", [128, CCH, HPC * HD], BF16, kind="ExternalInput").ap()
    wv_d = nc.dram_tensor("wv", [128, CCH, HPC * HD], BF16, kind="ExternalInput").ap()
    wp01_d = nc.dram_tensor("wp01", [128, DIM], BF16, kind="ExternalInput").ap()
    wp2_d = nc.dram_tensor("wp2", [128, DIM], BF16, kind="ExternalInput").ap()
    out_d = nc.dram_tensor("outp", [DIM, NQ], F32, kind="ExternalOutput").ap()

    with tile.TileContext(nc) as tc:
        with (
            tc.tile_pool(name="persist", bufs=1) as pp,
            tc.tile_pool(name="psA", bufs=1, space="PSUM") as psA,
            tc.tile_pool(name="psB", bufs=1, space="PSUM") as psB,
            tc.tile_pool(name="pvp", bufs=1, space="PSUM") as pvp,
            tc.tile_pool(name="bgp", bufs=2, space="PSUM") as bgp,
            tc.tile_pool(name="expp", bufs=17) as expp,
            tc.tile_pool(name="nrm", bufs=4) as nrm,
            tc.tile_pool(name="nrmd", bufs=3, space="DRAM") as nrmd,
            tc.tile_pool(name="outs", bufs=2) as outs,
        ):
            xw = pp.tile([128, CCH, N_TOK], BF16, tag="xw")
            qx = pp.tile([128, CCH, NQ], BF16, tag="qx")
            wk = pp.tile([128, CCH, HPC * HD], BF16, tag="wk")
            wq = pp.tile([128, CCH, HPC * HD], BF16, tag="wq")
            wv = pp.tile([128, CCH, HPC * HD], BF16, tag="wv")
            wp01 = pp.tile([128, DIM], BF16, tag="wp01")
            wp2 = pp.tile([128, DIM], BF16, tag="wp2")
            kTab = [
                pp.tile([128, NPAIR, 128], BF16, tag=f"kT{h}", name=f"kT{h}")
                for h in range(HPC)
            ]
            qTab = [
                pp.tile([128, QB, 512], BF16, tag=f"qT{h}", name=f"qT{h}")
                for h in range(HPC)
            ]
            v2 = [
                pp.tile([128, KCH, HD + 1], BF16, tag=f"v2_{h}", name=f"v2_{h}")
                for h in range(HPC)
            ]
            attT01 = pp.tile([128, QB, 512], BF16, tag="attT01")
            attT2ab = pp.tile([128, QB // 2, 512], BF16, tag="attT2ab")
            for h in range(HPC):
                nc.vector.memset(v2[h][:, :, HD : HD + 1], 1.0)
            onesb = pp.tile([HD + 1, HD], BF16, tag="onesb")
            nc.vector.memset(onesb, 1.0)
            warm = pp.tile([1, 2], F32, tag="warm")
            nc.vector.memset(warm, 0.0)
            nc.scalar.activation(
                out=warm, in_=warm, func=mybir.ActivationFunctionType.Exp
            )

            counter = [0]

            def uname(p):
                counter[0] += 1
                return f"{p}{counter[0]}"

            def emit_body(first=True):
                if first or reload_inputs:
                    emit_input_dmas()

            def emit_input_dmas():
                nc.sync.dma_start(out=wk, in_=wk_d)
                nc.sync.dma_start(out=xw[:, :, 0:512], in_=xw_d[:, :, 0:512])
                nc.sync.dma_start(out=wq, in_=wq_d)
                nc.sync.dma_start(out=qx[:, :, 0:512], in_=qx_d[:, :, 0:512])
                nc.sync.dma_start(out=wv, in_=wv_d)
                for t in range(1, 8):
                    sl = slice(512 * t, 512 * (t + 1))
                    nc.sync.dma_start(out=xw[:, :, sl], in_=xw_d[:, :, sl])
                for t in range(1, QB):
                    sl = slice(512 * t, 512 * (t + 1))
                    nc.sync.dma_start(out=qx[:, :, sl], in_=qx_d[:, :, sl])
                nc.sync.dma_start(out=wp01, in_=wp01_d)
                nc.sync.dma_start(out=wp2, in_=wp2_d)

            def emit_rest():
                def k_block(h, b):
                    """kTab[h] pairs 2b, 2b+1 (token chunks 4b..4b+3)."""
                    P = bgp.tile([128, 512], F32, tag="bg", name=uname("kp"))
                    sl = slice(512 * b, 512 * (b + 1))
                    wcol = slice(HD * h, HD * (h + 1))
                    for c in range(CCH):
                        blk = xw[:, c, sl].rearrange(
                            "p (a b k) -> p a b k", a=2, b=2
                        )
                        nc.tensor.matmul(
                            P[0:64, 0:256], wk[:, c, wcol], blk[:, :, 0, :],
                            start=(c == 0), stop=(c == CCH - 1),
                            skip_group_check=True,
                        )
                        nc.tensor.matmul(
                            P[64:128, 0:256], wk[:, c, wcol], blk[:, :, 1, :],
                            start=(c == 0), stop=(c == CCH - 1),
                            skip_group_check=True,
                        )
                    nc.vector.tensor_copy(
                        out=kTab[h][:, 2 * b : 2 * b + 2, :].rearrange(
                            "p a k -> p (a k)"
                        ),
                        in_=P[:, 0:256],
                    )

                def q_block01(b):
                    P = bgp.tile([128, 512], F32, tag="bg", name=uname("qp"))
                    sl = slice(512 * b, 512 * (b + 1))
                    for c in range(CCH):
                        nc.tensor.matmul(
                            P[0:64, :], wq[:, c, 0:HD], qx[:, c, sl],
                            start=(c == 0), stop=(c == CCH - 1),
                            skip_group_check=True,
                        )
                        nc.tensor.matmul(
                            P[64:128, :], wq[:, c, HD : 2 * HD], qx[:, c, sl],
                            start=(c == 0), stop=(c == CCH - 1),
                            skip_group_check=True,
                        )
                    nc.vector.tensor_copy(out=qTab[0][0:64, b, :], in_=P[0:64, :])
                    nc.vector.tensor_copy(
                        out=qTab[1][64:128, b, :], in_=P[64:128, :]
                    )
                    nc.sync.dma_start(
                        out=qTab[0][64:128, b, :], in_=qTab[0][0:64, b, :]
                    )
                    nc.sync.dma_start(
                        out=qTab[1][0:64, b, :], in_=qTab[1][64:128, b, :]
                    )

                def q_block2(b0, b1):
                    P = bgp.tile([128, 512], F32, tag="bg", name=uname("q2"))
                    s0 = slice(512 * b0, 512 * (b0 + 1))
                    s1 = slice(512 * b1, 512 * (b1 + 1))
                    wcol = slice(2 * HD, 3 * HD)
                    for c in range(CCH):
                        nc.tensor.matmul(
                            P[0:64, :], wq[:, c, wcol], qx[:, c, s0],
                            start=(c == 0), stop=(c == CCH - 1),
                            skip_group_check=True,
                        )
                        nc.tensor.matmul(
                            P[64:128, :], wq[:, c, wcol], qx[:, c, s1],
                            start=(c == 0), stop=(c == CCH - 1),
                            skip_group_check=True,
                        )
                    nc.vector.tensor_copy(out=qTab[2][0:64, b0, :], in_=P[0:64, :])
                    nc.vector.tensor_copy(
                        out=qTab[2][64:128, b1, :], in_=P[64:128, :]
                    )
                    nc.sync.dma_start(
                        out=qTab[2][64:128, b0, :], in_=qTab[2][0:64, b0, :]
                    )
                    nc.sync.dma_start(
                        out=qTab[2][0:64, b1, :], in_=qTab[2][64:128, b1, :]
                    )

                def v_pair(t2):
                    # two token chunks share one psum tile via sequential
                    # accumulation groups (WAR through the copies)
                    P = bgp.tile([128, 512], F32, tag="bg", name=uname("vp"))
                    for half in range(2):
                        t = 2 * t2 + half
                        tsl = slice(128 * t, 128 * (t + 1))
                        for c in range(CCH):
                            nc.tensor.matmul(
                                P[:, 0 : HPC * HD], xw[:, c, tsl],
                                wv[:, c, :],
                                start=(c == 0), stop=(c == CCH - 1),
                                skip_group_check=True,
                            )
                        for h in range(HPC):
                            nc.vector.tensor_copy(
                                out=v2[h][:, t, 0:HD],
                                in_=P[:, HD * h : HD * (h + 1)],
                            )

                background = deque()
                v_done = [0]

                def bg_v(t2):
                    def run():
                        v_pair(t2)
                        v_done[0] = 2 * (t2 + 1)
                    return ("v", run)

                for t2 in range(KCH // 2):
                    background.append(bg_v(t2))

                def tick(budget=1):
                    for _ in range(budget):
                        if not background:
                            return
                        kind, run = background[0]
                        if kind == "pv" and run.chunks_hi > v_done[0]:
                            for i, (k2, r2) in enumerate(background):
                                if k2 == "v":
                                    del background[i]
                                    r2()
                                    break
                            else:
                                return
                            continue
                        background.popleft()
                        if kind == "pv":
                            run.run()
                        else:
                            run()

                def auto_tick():
                    tick(2 if v_done[0] < KCH else 1)

                kblk = [0, 0, 0]

                def kq_jit(h, pairs_needed):
                    while kblk[h] * 2 < pairs_needed:
                        k_block(h, kblk[h])
                        kblk[h] += 1

                sweep_no = [0]  # global sweep counter for ACT/DVE split

                def attention_unit(h, qb, fast_fin=False):
                    # pv bank is shared (pvp bufs=1): all previous units'
                    # deferred pv work must drain before this unit's pv
                    while any(k in ("pv", "fin") for k, _ in background):
                        tick(3)
                    pv_box = [None]
                    unit_deferred = [False]

                    def get_pv():
                        if pv_box[0] is None:
                            pv_box[0] = pvp.tile(
                                [128, 512], F32, tag="pv", name=uname("pv")
                            )
                        return pv_box[0]

                    def pv_sweep(c0, nch, ex, is16):
                        pv = get_pv()
                        for j in range(nch):
                            c = c0 + j
                            rhs = ex[:, j, :]
                            if is16:
                                rhs = rhs.bitcast(BF16)
                            nc.tensor.matmul(
                                pv[0 : HD + 1, :],
                                v2[h][:, c, :],
                                rhs,
                                start=(c == 0),
                                stop=(c == KCH - 1),
                            )

                    class DefPV:
                        def __init__(self, c0, nch, ex, is16):
                            self.c0, self.nch, self.ex = c0, nch, ex
                            self.is16 = is16
                            self.chunks_hi = c0 + nch

                        def run(self):
                            pv_sweep(self.c0, self.nch, self.ex, self.is16)

                    # walk chunks in pair order; sweep boundaries from _SWEEPS
                    si = 0
                    cur_tile = None
                    for m in range(NPAIR):
                        kq_jit(h, m + 1)
                        for half in range(2):
                            c = 2 * m + half
                            c0, nch = _SWEEPS[si]
                            if c == c0:
                                pool = psA if nch == 3 else psB
                                cur_tile = pool.tile(
                                    [128, nch, 512], F32, tag="sc",
                                    name=uname("sw"),
                                )
                            rows = slice(64 * half, 64 * (half + 1))
                            nc.tensor.matmul(
                                cur_tile[:, c - c0, :],
                                kTab[h][rows, c // 2, :],
                                qTab[h][rows, qb, :],
                                start=True,
                                stop=True,
                            )
                            if c == c0 + nch - 1:
                                use_dve = (sweep_no[0] % dve_every) == 2
                                sweep_no[0] += 1
                                src = cur_tile[:, 0:nch, :].rearrange(
                                    "p a b -> p (a b)"
                                )
                                if use_dve:
                                    ex = expp.tile(
                                        [128, nch, 512], I16, tag="ex",
                                        name=uname("ex"),
                                    )
                                    nc.vector.tensor_scalar(
                                        out=ex[:, 0:nch, :].rearrange(
                                            "p a b -> p (a b)"
                                        ),
                                        in0=src,
                                        scalar1=float(SCH_A * SCALE),
                                        scalar2=float(SCH_B),
                                        op0=mybir.AluOpType.mult,
                                        op1=mybir.AluOpType.add,
                                    )
                                else:
                                    ex = expp.tile(
                                        [128, nch, 512], BF16, tag="ex",
                                        name=uname("ex"),
                                    )
                                    nc.scalar.activation(
                                        out=ex[:, 0:nch, :].rearrange(
                                            "p a b -> p (a b)"
                                        ),
                                        in_=src,
                                        func=mybir.ActivationFunctionType.Exp,
                                        scale=float(SCALE),
                                    )
                                if unit_deferred[0] or c0 + nch > v_done[0]:
                                    unit_deferred[0] = True
                                    background.append(
                                        ("pv", DefPV(c0, nch, ex, use_dve))
                                    )
                                else:
                                    pv_sweep(c0, nch, ex, use_dve)
                                si += 1
                                auto_tick()
                    if any(k == "pv" for k, _ in background):
                        background.append(
                            ("fin", lambda: finish(h, qb, get_pv(), True))
                        )
                    else:
                        finish(h, qb, get_pv(), True)

                def finish(h, qb, pv, fast=False):
                    rec = nrm.tile([HD + 1, 512], F32, tag="rec", name=uname("rc"))
                    nc.vector.reciprocal(
                        out=rec[HD : HD + 1, :], in_=pv[HD : HD + 1, :]
                    )
                    bc = nrm.tile([HD, 512], F32, tag="bc", name=uname("bc"))
                    if fast:
                        recb = nrm.tile(
                            [HD + 1, 512], BF16, tag="recb", name=uname("rb")
                        )
                        nc.vector.tensor_copy(
                            out=recb[HD : HD + 1, :], in_=rec[HD : HD + 1, :]
                        )
                        bcp = bgp.tile([128, 512], F32, tag="bg", name=uname("bq"))
                        nc.tensor.matmul(
                            bcp[0:HD, :], onesb[HD : HD + 1, :],
                            recb[HD : HD + 1, :],
                            start=True, stop=True,
                        )
                        nc.vector.tensor_copy(out=bc, in_=bcp[0:HD, :])
                    else:
                        recd = nrmd.tile([1, 512], F32, tag="recd", name=uname("rd"))
                        nc.sync.dma_start(out=recd, in_=rec[HD : HD + 1, :])
                        nc.sync.dma_start(
                            out=bc, in_=recd[:].to_broadcast((HD, 512))
                        )
                    if h == 0:
                        nc.vector.tensor_mul(
                            attT01[0:64, qb, :], pv[0:HD, :], bc
                        )
                    elif h == 1:
                        tmp = nrm.tile([HD, 512], BF16, tag="tmp", name=uname("tm"))
                        nc.vector.tensor_mul(tmp, pv[0:HD, :], bc)
                        nc.sync.dma_start(out=attT01[64:128, qb, :], in_=tmp)
                    else:
                        if qb % 2 == 0:
                            nc.vector.tensor_mul(
                                attT2ab[0:64, qb // 2, :], pv[0:HD, :], bc
                            )
                        else:
                            tmp = nrm.tile(
                                [HD, 512], BF16, tag="tmp", name=uname("tm")
                            )
                            nc.vector.tensor_mul(tmp, pv[0:HD, :], bc)
                            nc.sync.dma_start(
                                out=attT2ab[64:128, qb // 2, :], in_=tmp
                            )

                def proj_qm(qb, m):
                    def run():
                        pj = bgp.tile([128, 512], F32, tag="bg", name=uname("pj"))
                        nc.tensor.matmul(
                            pj, wp01[:, 128 * m : 128 * (m + 1)],
                            attT01[:, qb, :], start=True, stop=False,
                        )
                        rows = slice(64 * (qb % 2), 64 * (qb % 2) + 64)
                        nc.tensor.matmul(
                            pj, wp2[rows, 128 * m : 128 * (m + 1)],
                            attT2ab[rows, qb // 2, :],
                            start=False, stop=True,
                        )
                        ot = outs.tile([128, 512], F32, tag="ot", name=uname("ot"))
                        nc.vector.tensor_copy(out=ot, in_=pj)
                        nc.sync.dma_start(
                            out=out_d[128 * m : 128 * (m + 1),
                                      512 * qb : 512 * (qb + 1)],
                            in_=ot,
                        )
                    return ("proj", run)

                q_block01(0)
                q_block2(0, 1)
                attention_unit(0, 0)
                attention_unit(1, 0)
                q_block01(1)
                attention_unit(2, 0)
                attention_unit(2, 1)
                q_block01(2)
                q_block2(2, 3)
                attention_unit(0, 1)
                attention_unit(1, 1)
                for m in range(CCH):
                    background.append(proj_qm(0, m))
                    background.append(proj_qm(1, m))
                q_block01(3)
                attention_unit(0, 2)
                attention_unit(1, 2)
                attention_unit(2, 2)
                for m in range(CCH):
                    background.append(proj_qm(2, m))
                attention_unit(2, 3, fast_fin=True)
                attention_unit(0, 3, fast_fin=True)
                attention_unit(1, 3, fast_fin=True)
                while background:
                    tick(4)
                for m in range(CCH):
                    _, run = proj_qm(3, m)
                    run()

            for _rep in range(repeat):
                emit_body(first=(_rep == 0))
                emit_rest()

    nc.compile()
    return nc


def _prep_contraction_major(a_t: np.ndarray) -> np.ndarray:
    n = a_t.shape[1]
    return np.ascontiguousarray(
        a_t.reshape(CCH, 128, n).transpose(1, 0, 2).astype(NPBF16)
    )


def make_in_maps(x, w_qkv, w_proj):
    xw_np = _prep_contraction_major(x.T)
    qx_np = [
        _prep_contraction_major(np.ascontiguousarray(x[NQ * s : NQ * (s + 1)].T))
        for s in range(2)
    ]
    in_maps = []
    for c in range(N_CORES):
        g, s = c >> 1, c & 1
        heads = [3 * g + hh for hh in range(HPC)]
        wq_g = np.concatenate([w_qkv[HD * h : HD * (h + 1)] for h in heads])
        wk_g = np.concatenate(
            [w_qkv[DIM + HD * h : DIM + HD * (h + 1)] for h in heads]
        )
        wv_g = np.concatenate(
            [w_qkv[2 * DIM + HD * h : 2 * DIM + HD * (h + 1)] for h in heads]
        )
        wp_h = [
            np.ascontiguousarray(w_proj[:, HD * h : HD * (h + 1)].T)
            for h in heads
        ]
        wp01 = np.concatenate([wp_h[0], wp_h[1]]).astype(NPBF16)
        wp2 = np.concatenate([wp_h[2], wp_h[2]]).astype(NPBF16)
        in_maps.append(
            {
                "xw": xw_np,
                "qx": qx_np[s],
                "wk": _prep_contraction_major(np.ascontiguousarray(wk_g.T)),
                "wq": _prep_contraction_major(np.ascontiguousarray(wq_g.T)),
                "wv": _prep_contraction_major(np.ascontiguousarray(wv_g.T)),
                "wp01": wp01,
                "wp2": wp2,
            }
        )
    return in_maps


def kernel(x, w_qkv, w_proj, b_proj):
    x = np.asarray(x, dtype=np.float32)
    w_qkv = np.asarray(w_qkv, dtype=np.float32)
    w_proj = np.asarray(w_proj, dtype=np.float32)
    b_proj = np.asarray(b_proj, dtype=np.float32)

    if "nc" not in _cache:
        _cache["nc"] = _build_program()
    nc = _cache["nc"]

    in_maps = make_in_maps(x, w_qkv, w_proj)

    try:
        res = bass_utils.run_bass_kernel_spmd(
            nc,
            in_maps,
            core_ids=list(range(N_CORES)),
            trace=bool(os.environ.get("KERNEL_TRACE")),
        )
    except ModuleNotFoundError:
        os.environ["BASS_NEVER_TRACE"] = "1"
        res = bass_utils.run_bass_kernel_spmd(
            nc, in_maps, core_ids=list(range(N_CORES))
        )
    kernel.last_results = res

    out = np.tile(b_proj.astype(np.float32), (N_TOK, 1))
    for c in range(N_CORES):
        s = c & 1
        out[NQ * s : NQ * (s + 1)] += res.results[c]["outp"].T
    return out

